# revision 10
# baseline (speedup 1.0000x reference)
"""Trainium2 Bass kernel for nn_ConvGraph_SC (gnn_message_passing).

Reference computation (per batch b of 64, N=32 nodes, C=512 channels, 7x7 spatial):
    state = input.mean(axis=(3,4))                       # [B, N, C]
    mat1  = state @ W1.T + b1
    mat2  = state @ W2.T + b2
    adj   = mat1 @ mat2.T                                # [B, N, N]
    soft  = softmax((adj - mean(adj)) / std(adj), rows)  # global mean/std, ddof=1
    out   = mean(soft @ state + state, axis=1)           # [B, C]

Device-side algebra (S = spatial SUM of x, unscaled):
  * softmax((adj-mu)/sigma) is invariant to a per-batch affine rescale of adj,
    so we work with adj' = 2401*adj = S A S^T + 49*s_u 1^T + 49*1 s_v^T + c0'
    where A = W1^T W2 (UNSCALED -> fp16-friendly magnitudes), u' = 49 W1^T b2,
    v' = 49 W2^T b1, c0' = 2401 b1.b2.
  * Row-constant terms (s_u, c0) drop out of the row softmax; they enter only
    the global mean/std, computed from per-row sums with closed-form
    corrections.
  * fp16 datapath on the PE (1 cycle/row vs 4 for fp32): state sums, A, u, v
    in fp16; PSUM accumulation stays fp32. rel-err budget is 2e-2; this lands
    ~1e-3.
  * Engine balance: the 52us of spatial-sum reduction cannot live on DVE
    alone. The scalar engine converts most x tiles f32->fp16 (idle otherwise)
    and DVE reduces fp16 at 2 elem/cycle (2x_1p mode). GpSimd takes the
    PSUM->SBUF copies.
  * Epilogue out[c] = sum_k (colsum(soft)[k]+1)/ (N*HW) * S[k,c] runs on the
    PE: stationary wf4[p,r] = w[p//4]*(p%4==r) against moving sraw -> [4,128]
    natural output.
  * Last batch is streamed as 4 quarter-DMAs reduced immediately to shrink
    the post-stream tail.

Sharding: pure data parallel, 8 batches per NeuronCore, weights replicated.
"""

import numpy as np

import concourse.bacc as bacc
import concourse.tile as tile
from concourse import masks, mybir
from concourse.bass_utils import run_bass_kernel_spmd

F32 = mybir.dt.float32
F16 = mybir.dt.float16
I32 = mybir.dt.int32
NCORES = 8
B, N, C, HW = 64, 32, 512, 49
BPC = B // NCORES          # batches per core
FREE = N * C * HW // 128   # 6272 floats per partition per batch
HALF = FREE // 2           # 3136
QUAR = FREE // 4           # 1568
G = 4                      # batches per stats group
NG = BPC // G              # groups per core
K1023 = float(np.sqrt(np.float64(1023.0)))
# halves (i = 2b + h) reduced directly from f32 on DVE; the rest are converted
# to fp16 on the scalar engine first. Batch 7 streams as quarters (all direct).
DIRECT = {0, 1}

_CACHED_NC = None

A_ = mybir.AluOpType


def build_bass():
    nc = bacc.Bacc("TRN2", target_bir_lowering=False)

    x_d = nc.declare_dram_parameter("x", [BPC, 128, FREE], F32, isOutput=False)
    a_d = nc.declare_dram_parameter("amat", [C, C], F16, isOutput=False)
    uv_d = nc.declare_dram_parameter("uv", [C, 2], F16, isOutput=False)
    c0_d = nc.declare_dram_parameter("c0", [32, 1], F32, isOutput=False)
    msk_d = nc.declare_dram_parameter("msk", [128, 36], F16, isOutput=False)
    out_d = nc.declare_dram_parameter("out", [4, 128 * BPC], F32, isOutput=True)

    lp = nc.allow_low_precision("fp16 spatial sums; rel-err budget 2e-2")
    lp.__enter__()
    with tile.TileContext(nc) as tc:
        with (
            tc.tile_pool(name="xpool", bufs=7) as xpool,
            tc.tile_pool(name="xqpool", bufs=4) as xqpool,
            tc.tile_pool(name="xhpool", bufs=4) as xhpool,
            tc.tile_pool(name="singles", bufs=1) as singles,
            tc.tile_pool(name="tap", bufs=2) as tap,
            tc.tile_pool(name="small", bufs=2) as small,
            tc.tile_pool(name="ps_t", bufs=1, space="PSUM") as ps_t_pool,
            tc.tile_pool(name="ps_tt", bufs=2, space="PSUM") as ps_tt_pool,
            tc.tile_pool(name="ps_adj", bufs=2, space="PSUM") as ps_adj_pool,
            tc.tile_pool(name="ps_misc", bufs=2, space="PSUM") as ps_misc_pool,
            tc.tile_pool(name="ps_out", bufs=1, space="PSUM") as ps_out_pool,
        ):
            # ---- persistent tiles -----------------------------------------
            ident = singles.tile([128, 128], F16)
            ones16 = singles.tile([1, 128], F16)
            ones_col = singles.tile([32, 1], F32)
            ones_r32 = singles.tile([1, 32], F32)
            a_sb = singles.tile([128, 4 * C], F16)
            uv_sb = singles.tile([128, 8], F16)
            c0_sb = singles.tile([32, 1], F32)
            msk_sb = singles.tile([128, 36], F16)  # [:, :32]=M32, [:, 32:36]=mask4
            # sraw: spatial sums, natural layout: [p=(4n+c_hi), c_low], fp16
            sraw_all = singles.tile([128, 128 * BPC], F16)
            # state^T: [p=c_low, 128b + 4n + c_hi], fp16
            st_all = singles.tile([128, 128 * BPC], F16)
            outsb = singles.tile([4, 128 * BPC], F32)

            def load_weights():
                # emitted after the first batch's x DMAs so the input stream
                # owns the head of the DMA queues
                for r in range(4):
                    nc.sync.dma_start(
                        out=a_sb[:, 512 * r : 512 * (r + 1)],
                        in_=a_d[128 * r : 128 * (r + 1), :],
                    )
                for r in range(4):
                    nc.sync.dma_start(
                        out=uv_sb[:, 2 * r : 2 * (r + 1)],
                        in_=uv_d[128 * r : 128 * (r + 1), :],
                    )
                nc.sync.dma_start(out=c0_sb[:], in_=c0_d[:])
                nc.sync.dma_start(out=msk_sb[:], in_=msk_d[:])
                masks.make_identity(nc, ident[:])
                nc.gpsimd.memset(ones16[:], 1.0)
                nc.gpsimd.memset(ones_col[:], 1.0)
                nc.gpsimd.memset(ones_r32[:], 1.0)

            # per-group state (allocated lazily in program order)
            grp = {}

            def start_group(g):
                grp[g] = {
                    # adj [:32, 0:128]; sv rows [0:1, 128+32bp : 160+32bp]
                    "ps_adj": ps_adj_pool.tile([32, 256], F32, name="ps_adj"),
                    # ps_misc regions: su cols [:32, 0:4], stats colsum
                    # [:1, 8:16], stats bcast [:32, 16:24], wf colsums
                    # [:1, 64+32bp], wfb bcast [:, 192+32bp]
                    "ps_misc": ps_misc_pool.tile([128, 512], F32, name="ps_misc"),
                    "ps_out": ps_out_pool.tile([4, 512], F32, name="ps_out"),
                    "sv": small.tile([1, 128], F16, tag="sv", name="sv"),
                    "q": small.tile([32, G], F32, tag="q", name="q"),
                    "t": small.tile([32, G], F32, tag="t", name="t"),
                    "rq": small.tile([32, G], F32, tag="rq", name="rq"),
                    "nm": small.tile([32, G], F32, tag="nm", name="nm"),
                    "expt": small.tile([32, 32 * G], F32, tag="expt", name="expt"),
                    "rowsum": small.tile([32, G], F32, tag="rowsum", name="rowsum"),
                }

            def per_batch(b):
                g, bp = divmod(b, G)
                if bp == 0:
                    start_group(g)
                gd = grp[g]
                scol = slice(128 * b, 128 * (b + 1))

                # -- load + spatial sum (fp16) --------------------------------
                if b < BPC - 1:
                    for h in range(2):
                        i = 2 * b + h
                        xb = xpool.tile([128, HALF], F32, tag="xb")
                        nc.sync.dma_start(
                            out=xb[:], in_=x_d[b, :, HALF * h : HALF * (h + 1)]
                        )
                        dst = sraw_all[:, 128 * b + 64 * h : 128 * b + 64 * (h + 1)]
                        if i in DIRECT:
                            nc.vector.reduce_sum(
                                out=dst,
                                in_=xb[:].rearrange("p (q s) -> p q s", s=HW),
                                axis=mybir.AxisListType.X,
                            )
                        else:
                            xh = xhpool.tile([128, HALF], F16, tag="xh")
                            nc.scalar.copy(xh[:], xb[:])
                            nc.vector.reduce_sum(
                                out=dst,
                                in_=xh[:].rearrange("p (q s) -> p q s", s=HW),
                                axis=mybir.AxisListType.X,
                            )
                    if b == 0:
                        load_weights()
                    # transpose sraw -> st (fp16, via PE)
                    ps_t = ps_t_pool.tile([128, 128], F16)
                    nc.tensor.transpose(ps_t[:], sraw_all[:, scol], ident[:])
                    nc.scalar.copy(st_all[:, scol], ps_t[:])
                else:
                    # last batch: quarters, direct-reduced, half-transposes
                    ps_t = ps_t_pool.tile([128, 128], F16)
                    for qi in range(4):
                        xb = xqpool.tile([128, QUAR], F32, tag="xbq")
                        nc.sync.dma_start(
                            out=xb[:], in_=x_d[b, :, QUAR * qi : QUAR * (qi + 1)]
                        )
                        nc.vector.reduce_sum(
                            out=sraw_all[
                                :, 128 * b + 32 * qi : 128 * b + 32 * (qi + 1)
                            ],
                            in_=xb[:].rearrange("p (q s) -> p q s", s=HW),
                            axis=mybir.AxisListType.X,
                        )
                        if qi % 2 == 1:
                            hh = qi // 2
                            nc.tensor.transpose(
                                ps_t[64 * hh : 64 * (hh + 1), :],
                                sraw_all[
                                    :, 128 * b + 64 * hh : 128 * b + 64 * (hh + 1)
                                ],
                                ident[:],
                            )
                    nc.scalar.copy(st_all[:, scol], ps_t[:])

                def st_slice(r):
                    return st_all[:, 128 * b + r : 128 * (b + 1) : 4]

                # -- TA^T = A^T S^T directly: [p=d_low, 32s+k] ---------------
                ps_tt = ps_tt_pool.tile([128, 128], F32)
                for s in range(4):
                    for r in range(4):
                        nc.tensor.matmul(
                            ps_tt[:, 32 * s : 32 * (s + 1)],
                            a_sb[:, 512 * r + 128 * s : 512 * r + 128 * (s + 1)],
                            st_slice(r),
                            start=(r == 0), stop=(r == 3),
                        )
                ta_b = tap.tile([128, 128], F16, tag="ta")
                nc.scalar.copy(ta_b[:], ps_tt[:])

                # -- su column + sv row --------------------------------------
                ps_misc = gd["ps_misc"]
                for r in range(4):
                    nc.tensor.matmul(
                        ps_misc[:32, bp : bp + 1],
                        st_slice(r),
                        uv_sb[:, 2 * r : 2 * r + 1],
                        start=(r == 0), stop=(r == 3),
                    )
                svsl = slice(128 + 32 * bp, 128 + 32 * (bp + 1))
                for r in range(4):
                    nc.tensor.matmul(
                        gd["ps_adj"][0:1, svsl],
                        uv_sb[:, 2 * r + 1 : 2 * r + 2],
                        st_slice(r),
                        start=(r == 0), stop=(r == 3),
                    )
                nc.vector.tensor_copy(
                    gd["sv"][:, 32 * bp : 32 * (bp + 1)], gd["ps_adj"][0:1, svsl]
                )

                # -- adjacency': TA S^T + 1 sv^T (minus row-constants) -------
                ps_adj = gd["ps_adj"]
                asl = slice(32 * bp, 32 * (bp + 1))
                for s in range(4):
                    nc.tensor.matmul(
                        ps_adj[:, asl],
                        ta_b[:, 32 * s : 32 * (s + 1)],
                        st_slice(s),
                        start=(s == 0), stop=False,
                    )
                nc.tensor.matmul(
                    ps_adj[:, asl],
                    ones16[0:1, 0:32],
                    gd["sv"][0:1, 32 * bp : 32 * (bp + 1)],
                    start=False, stop=True,
                )

                # -- per-batch stats pieces ----------------------------------
                nc.vector.reduce_sum(
                    out=gd["t"][:, bp : bp + 1], in_=ps_adj[:, asl],
                    axis=mybir.AxisListType.X,
                )
                sq_scr = small.tile([32, 32], F32, tag="sq_scr")
                nc.scalar.activation(
                    out=sq_scr[:], in_=ps_adj[:, asl],
                    func=mybir.ActivationFunctionType.Square,
                    accum_out=gd["rq"][:, bp : bp + 1],
                )
                nc.vector.reduce_max(
                    out=gd["nm"][:, bp : bp + 1], in_=ps_adj[:, asl],
                    axis=mybir.AxisListType.X, negate=True,
                )
                nc.vector.tensor_scalar(
                    out=gd["q"][:, bp : bp + 1], in0=ps_misc[:32, bp : bp + 1],
                    scalar1=c0_sb[:], scalar2=None, op0=A_.add,
                )

            def finish_group(g):
                gd = grp[g]
                ps_adj, ps_misc, ps_out = gd["ps_adj"], gd["ps_misc"], gd["ps_out"]
                q_g, t_g, rowsq = gd["q"], gd["t"], gd["rq"]

                # ---- stats: S1/S2 of TRUE adj' via row sums ----------------
                # stats_g: cols 0:G = S1 rows, G:2G = S2 rows
                stats_g = small.tile([32, 2 * G], F32, tag="stats_g")
                q32 = small.tile([32, G], F32, tag="q32")
                nc.vector.tensor_scalar(
                    out=q32[:], in0=q_g[:], scalar1=32.0, scalar2=None, op0=A_.mult,
                )
                nc.vector.tensor_add(stats_g[:, 0:G], q32[:], t_g[:])
                # S2row = rowsq + q*(2t + 32q); 2t + 32q = t + S1row
                h_g = small.tile([32, G], F32, tag="h_g")
                nc.vector.tensor_add(h_g[:], t_g[:], stats_g[:, 0:G])
                s2c = small.tile([32, G], F32, tag="s2c")
                nc.vector.tensor_mul(s2c[:], q_g[:], h_g[:])
                nc.vector.tensor_add(stats_g[:, G : 2 * G], rowsq[:], s2c[:])

                # cross-partition sum + broadcast back (PE ones trick)
                nc.tensor.matmul(
                    ps_misc[:1, 8:16], ones_col[:], stats_g[:],
                    start=True, stop=True,
                )
                s_sb = small.tile([1, 2 * G], F32, tag="s_sb")
                nc.vector.tensor_copy(s_sb[:], ps_misc[:1, 8:16])
                nc.tensor.matmul(
                    ps_misc[:32, 16:24], ones_r32[:], s_sb[:],
                    start=True, stop=True,
                )
                s_all = small.tile([32, 2 * G], F32, tag="s_all")
                nc.vector.tensor_copy(s_all[:], ps_misc[:32, 16:24])

                # ---- inv_std = sqrt(1023)/sqrt(S2 - S1^2/1024) -------------
                t1 = small.tile([32, G], F32, tag="t1")
                nc.vector.tensor_mul(t1[:], s_all[:, 0:G], s_all[:, 0:G])
                nc.vector.tensor_scalar(
                    out=t1[:], in0=t1[:], scalar1=-1.0 / 1024.0, scalar2=None,
                    op0=A_.mult,
                )
                v1023 = small.tile([32, G], F32, tag="v1023")
                nc.vector.tensor_add(v1023[:], t1[:], s_all[:, G : 2 * G])
                # Newton rsqrt with magic seed (exp is the only table the
                # scalar engine needs)
                yint = small.tile([32, G], I32, tag="yint")
                nc.vector.tensor_scalar(
                    out=yint[:], in0=v1023[:].bitcast(I32), scalar1=1,
                    scalar2=None, op0=A_.logical_shift_right,
                )
                nc.vector.tensor_scalar(
                    out=yint[:], in0=yint[:], scalar1=-1,
                    scalar2=0x5F3759DF, op0=A_.mult, op1=A_.add,
                )
                y = small.tile([32, G], F32, tag="y")
                nc.vector.tensor_copy(y[:], yint[:].bitcast(F32))
                ya = small.tile([32, G], F32, tag="ya")
                yb = small.tile([32, G], F32, tag="yb")
                for it in range(3):
                    nc.vector.tensor_mul(ya[:], y[:], y[:])
                    nc.vector.tensor_mul(yb[:], ya[:], v1023[:])
                    last = it == 2
                    nc.vector.tensor_scalar(
                        out=ya[:], in0=yb[:],
                        scalar1=(-0.5 * K1023) if last else -0.5,
                        scalar2=(1.5 * K1023) if last else 1.5,
                        op0=A_.mult, op1=A_.add,
                    )
                    nc.vector.tensor_mul(y[:], y[:], ya[:])
                inv_g = y  # [32, G] inv_std per batch column

                # ---- softmax + epilogue ------------------------------------
                negm = small.tile([32, G], F32, tag="negm")
                nc.vector.tensor_mul(negm[:], gd["nm"][:], inv_g[:])
                expt, rowsum = gd["expt"], gd["rowsum"]
                for bp in range(G):
                    nc.scalar.activation(
                        out=expt[:, 32 * bp : 32 * (bp + 1)],
                        in_=ps_adj[:, 32 * bp : 32 * (bp + 1)],
                        func=mybir.ActivationFunctionType.Exp,
                        bias=negm[:, bp : bp + 1], scale=inv_g[:, bp : bp + 1],
                        accum_out=rowsum[:, bp : bp + 1],
                    )
                recip = small.tile([32, G], F32, tag="recip")
                nc.vector.reciprocal(recip[:], rowsum[:])

                # w[k] = colsum(soft) per batch -> [1, 32] rows at partition 0
                for bp in range(G):
                    nc.tensor.matmul(
                        ps_misc[:1, 64 + 32 * bp : 64 + 32 * (bp + 1)],
                        recip[:, bp : bp + 1],
                        expt[:, 32 * bp : 32 * (bp + 1)],
                        start=True, stop=True,
                    )
                wf16 = small.tile([1, 128], F16, tag="wf16")
                nc.vector.tensor_scalar(
                    out=wf16[:], in0=ps_misc[:1, 64:192],
                    scalar1=1.0 / (N * HW), scalar2=1.0 / (N * HW),
                    op0=A_.mult, op1=A_.add,
                )
                for bp in range(G):
                    b = G * g + bp
                    # broadcast wf to all partitions, then gather the diagonal
                    # wfb[p, p//4] and spread to wf4[p, r] = w[p//4]*(p%4==r)
                    wsl = slice(192 + 32 * bp, 224 + 32 * bp)
                    nc.tensor.matmul(
                        ps_misc[:, wsl],
                        ones16[:],
                        wf16[0:1, 32 * bp : 32 * (bp + 1)],
                        start=True, stop=True,
                    )
                    scr32 = small.tile([128, 32], F32, tag="scr32")
                    dcol = small.tile([128, 1], F32, tag="dcol")
                    nc.vector.scalar_tensor_tensor(
                        out=scr32[:], in0=ps_misc[:, wsl], scalar=1.0,
                        in1=msk_sb[:, 0:32], op0=A_.mult, op1=A_.mult,
                        accum_out=dcol[:],
                    )
                    wf4 = small.tile([128, 4], F16, tag="wf4")
                    nc.gpsimd.tensor_scalar(
                        out=wf4[:], in0=msk_sb[:, 32:36], scalar1=dcol[:],
                        scalar2=None, op0=A_.mult,
                    )
                    nc.tensor.matmul(
                        ps_out[:4, 128 * bp : 128 * (bp + 1)],
                        wf4[:],
                        sraw_all[:, 128 * b : 128 * (b + 1)],
                        start=True, stop=True,
                    )
                gsl = slice(512 * g, 512 * (g + 1))
                nc.scalar.copy(outsb[:, gsl], ps_out[:])
                nc.sync.dma_start(out=out_d[:, gsl], in_=outsb[:, gsl])

            # schedule: delay each group's reduction chain two batches so the
            # scalar/vector queues never head-of-line-block the next group's
            # stream processing
            for b in range(6):
                per_batch(b)
            finish_group(0)
            per_batch(6)
            per_batch(7)
            finish_group(1)

    lp.__exit__(None, None, None)
    nc.finalize()
    return nc


def host_prep(input, W1, b1, W2, b2):
    input = np.ascontiguousarray(input, dtype=np.float32)
    w1 = np.asarray(W1, dtype=np.float64)
    w2 = np.asarray(W2, dtype=np.float64)
    b1 = np.asarray(b1, dtype=np.float64)
    b2 = np.asarray(b2, dtype=np.float64)
    # softmax((adj-mu)/sigma) is scale-invariant per batch: use 2401*adj so A
    # stays in fp16-normal range
    amat = np.ascontiguousarray(w1.T @ w2, dtype=np.float16)
    u = HW * (w1.T @ b2)
    v = HW * (w2.T @ b1)
    uv = np.ascontiguousarray(np.stack([u, v], axis=1), dtype=np.float16)
    c0 = np.full((32, 1), float(HW * HW * (b1 @ b2)), dtype=np.float32)
    p = np.arange(128)
    m32 = (np.arange(32)[None, :] == (p[:, None] // 4)).astype(np.float16)
    m4 = (np.arange(4)[None, :] == (p[:, None] % 4)).astype(np.float16)
    msk = np.ascontiguousarray(np.concatenate([m32, m4], axis=1))
    return input, amat, uv, c0, msk


def make_in_maps(input, W1, b1, W2, b2):
    input, amat, uv, c0, msk = host_prep(input, W1, b1, W2, b2)
    in_maps = []
    for i in range(NCORES):
        shard = input[BPC * i : BPC * (i + 1)].reshape(BPC, 128, FREE)
        in_maps.append(
            {"x": shard, "amat": amat, "uv": uv, "c0": c0, "msk": msk}
        )
    return in_maps


def kernel(input, W1, b1, W2, b2):
    global _CACHED_NC
    if _CACHED_NC is None:
        _CACHED_NC = build_bass()
    nc = _CACHED_NC

    in_maps = make_in_maps(input, W1, b1, W2, b2)
    res = run_bass_kernel_spmd(nc, in_maps, list(range(NCORES)))

    out = np.empty((B, C), dtype=np.float32)
    for i in range(NCORES):
        o = res.results[i]["out"]  # [4, 128*BPC], out[b, 128r+q] = o[r, 128b+q]
        out[BPC * i : BPC * (i + 1)] = (
            o.reshape(4, BPC, 128).transpose(1, 0, 2).reshape(BPC, C)
        )
    return out
